# revision 2
# baseline (speedup 1.0000x reference)
"""Trainium2 Bass kernel for nn_MessagePassingGNN (B=8192 graphs, N=9 nodes,
16 edges + 9 self-loops per graph, 4 message-passing steps + GRU, decoder).

Strategy:
  - Data-parallel over batch: each of 8 cores gets 1024 graphs.
  - Within a core, graphs are packed into blocks of 14 (126 nodes, 350 edges)
    plus one tail block of 2 graphs. Gather (x[dst], x[src]) and scatter
    (mean-aggregation) are done as matmuls against host-precomputed one-hot
    incidence matrices, so the whole step pipeline lives on the TensorEngine.
  - Activations are kept transposed ([feat, rows]) so MLP layers chain without
    transposes; the only transpose is x^T -> X_row once per block-step for the
    gather stationary operand (done via DMA transpose).
  - All matmuls in bf16 (fp32 PSUM accumulation). Sigmoid is rewritten via
    tanh so the ScalarEngine needs a single LUT table set for the whole kernel.
  - deg-normalization is folded into the scatter one-hot; msg_b3 is folded
    into the GRU input bias (host-side), so no partition-broadcast is needed.
"""

import numpy as np

try:
    import concourse.bass as bass  # noqa: F401
except Exception:  # pragma: no cover
    import sys

    sys.path.insert(0, "/opt/trn_rl_repo")

import ml_dtypes
import concourse.bass as bass
import concourse.bacc as bacc
import concourse.mybir as mybir
from concourse.bass import MemorySpace
from concourse.bass_utils import run_bass_kernel_spmd
from concourse.tile import TileContext

BF16 = mybir.dt.bfloat16
F32 = mybir.dt.float32
NPBF16 = ml_dtypes.bfloat16
AF = mybir.ActivationFunctionType
ALU = mybir.AluOpType

N, F_IN, H, MH, STEPS = 9, 15, 128, 256, 4
E_PER = 16
EPG = E_PER + N  # 25 edges per graph incl self-loops
NCORES = 8
GPB = 14  # graphs per full block
NN = GPB * N  # 126 nodes per full block
NE = GPB * EPG  # 350 edges per full block

# bias-pack column map
COL_ENC = 0
COL_B1 = lambda s, c: 1 + 2 * s + c
COL_B2 = lambda s, c: 9 + 2 * s + c
COL_BRZ = lambda s, g: 17 + 2 * s + g  # 0.5*(bi'+bh)[g*128:+128]
COL_BHN = lambda s: 25 + s  # bh[256:384]
COL_BIN = lambda s: 29 + s  # bi'[256:384]
COL_DB1 = lambda c: 33 + c
COL_DB2 = lambda c: 35 + c
COL_DB3 = 37
NBIAS = 38


def _derive(bg):
    nblk = bg // GPB
    tailg = bg - nblk * GPB
    totblk = nblk + (1 if tailg else 0)
    nnode = bg * N
    return dict(bg=bg, nblk=nblk, tailg=tailg, totblk=totblk, nnode=nnode)


CFG_FULL = _derive(1024)

_NC_CACHE = {}


def build_nc(cfg):
    key = cfg["bg"]
    if key in _NC_CACHE:
        return _NC_CACHE[key]
    nblk, tailg, totblk, nnode = (
        cfg["nblk"],
        cfg["tailg"],
        cfg["totblk"],
        cfg["nnode"],
    )
    tnn, tne = tailg * N, tailg * EPG

    nc = bacc.Bacc("TRN2", target_bir_lowering=False, debug=False, num_devices=NCORES)

    obsT_d = nc.dram_tensor("obsT", [F_IN, nnode], BF16, kind="ExternalInput")
    sdt_d = nc.dram_tensor("sdt", [totblk, NN, 2 * NE], BF16, kind="ExternalInput")
    dwt_d = nc.dram_tensor("dwt", [totblk, 3, 128, NN], BF16, kind="ExternalInput")
    encw_d = nc.dram_tensor("encw", [F_IN, H], BF16, kind="ExternalInput")
    w1_d = nc.dram_tensor("w1", [STEPS, 2 * H, MH], BF16, kind="ExternalInput")
    w2_d = nc.dram_tensor("w2", [STEPS, MH, MH], BF16, kind="ExternalInput")
    w3_d = nc.dram_tensor("w3", [STEPS, MH, H], BF16, kind="ExternalInput")
    wi_d = nc.dram_tensor("wi", [STEPS, H, 3 * H], BF16, kind="ExternalInput")
    wh_d = nc.dram_tensor("wh", [STEPS, H, 3 * H], BF16, kind="ExternalInput")
    dw1_d = nc.dram_tensor("dw1", [H, MH], BF16, kind="ExternalInput")
    dw2_d = nc.dram_tensor("dw2", [MH, MH], BF16, kind="ExternalInput")
    dw3_d = nc.dram_tensor("dw3", [MH, 1], BF16, kind="ExternalInput")
    bias_d = nc.dram_tensor("biases", [128, NBIAS], F32, kind="ExternalInput")
    out_d = nc.dram_tensor("out", [1, nnode], F32, kind="ExternalOutput")

    with TileContext(nc) as tc:
        with tc.tile_pool(name="const", bufs=1) as constp:
            obs_t = constp.tile([F_IN, nnode], BF16, tag="obs")
            nc.sync.dma_start(obs_t[:], obsT_d[:])
            encw_t = constp.tile([F_IN, H], BF16, tag="encw")
            nc.sync.dma_start(encw_t[:], encw_d[:])
            w1_t = constp.tile([128, STEPS, 2, MH], BF16, tag="w1")
            nc.sync.dma_start(
                w1_t[:], w1_d.rearrange("s (kc p) m -> p s kc m", p=128)
            )
            w2_t = constp.tile([128, STEPS, 2, MH], BF16, tag="w2")
            nc.sync.dma_start(
                w2_t[:], w2_d.rearrange("s (kc p) m -> p s kc m", p=128)
            )
            w3_t = constp.tile([128, STEPS, 2, H], BF16, tag="w3")
            nc.sync.dma_start(
                w3_t[:], w3_d.rearrange("s (kc p) m -> p s kc m", p=128)
            )
            wi_t = constp.tile([128, STEPS, 3 * H], BF16, tag="wi")
            nc.sync.dma_start(wi_t[:], wi_d.rearrange("s p m -> p s m"))
            wh_t = constp.tile([128, STEPS, 3 * H], BF16, tag="wh")
            nc.sync.dma_start(wh_t[:], wh_d.rearrange("s p m -> p s m"))
            dw1_t = constp.tile([128, MH], BF16, tag="dw1")
            nc.sync.dma_start(dw1_t[:], dw1_d[:])
            dw2_t = constp.tile([128, 2, MH], BF16, tag="dw2")
            nc.sync.dma_start(dw2_t[:], dw2_d.rearrange("(kc p) m -> p kc m", p=128))
            dw3_t = constp.tile([128, 2, 1], BF16, tag="dw3")
            nc.sync.dma_start(dw3_t[:], dw3_d.rearrange("(kc p) m -> p kc m", p=128))
            bias_t = constp.tile([128, NBIAS], F32, tag="bias")
            nc.sync.dma_start(bias_t[:], bias_d[:])

            xb = [
                constp.tile([128, nnode], BF16, tag="xb0", name="xb0"),
                constp.tile([128, nnode], BF16, tag="xb1", name="xb1"),
            ]
            outsb = constp.tile([1, nnode], F32, tag="outsb")

            def bcol(c):
                return bias_t[:, c : c + 1]

            # ---------------- encoder ----------------
            with tc.tile_pool(name="encps", bufs=2, space=MemorySpace.PSUM) as encps:
                for t0 in range(0, nnode, 512):
                    w = min(512, nnode - t0)
                    ps = encps.tile([128, 512], F32, tag="encps")
                    nc.tensor.matmul(
                        ps[:, :w], encw_t[:, :], obs_t[:, t0 : t0 + w],
                        start=True, stop=True,
                    )
                    nc.scalar.activation(
                        xb[0][:, t0 : t0 + w], ps[:, :w], AF.Tanh, bias=bcol(COL_ENC)
                    )

            # ---------------- message-passing blocks ----------------
            with (
                tc.tile_pool(name="sd", bufs=3) as sdp,
                tc.tile_pool(name="dw", bufs=3) as dwp,
                tc.tile_pool(name="xrow", bufs=2) as xrowp,
                tc.tile_pool(name="eact", bufs=2) as eactp,
                tc.tile_pool(name="gact", bufs=2) as gactp,
                tc.tile_pool(name="ps350", bufs=3, space=MemorySpace.PSUM) as pp350,
                tc.tile_pool(name="psm3", bufs=2, space=MemorySpace.PSUM) as ppm3,
                tc.tile_pool(name="psagg", bufs=1, space=MemorySpace.PSUM) as ppagg,
                tc.tile_pool(name="psgru", bufs=2, space=MemorySpace.PSUM) as ppgru,
            ):
                for k in range(cfg["totblk"]):
                    full = k < nblk
                    nn = NN if full else tnn
                    ne = NE if full else tne
                    col0 = NN * k
                    cols = slice(col0, col0 + nn)
                    if full:
                        ecs = [(0, 128), (128, 128), (256, 94)]
                    else:
                        ecs = [(0, tne)]

                    sd = sdp.tile([NN, 2 * NE], BF16, tag="sd")
                    if full:
                        nc.sync.dma_start(sd[:, :], sdt_d[k])
                    else:
                        nc.sync.dma_start(sd[:nn, :ne], sdt_d[k, :nn, :ne])
                        nc.sync.dma_start(
                            sd[:nn, NE : NE + ne], sdt_d[k, :nn, NE : NE + ne]
                        )
                    dwti = dwp.tile([128, 3, NN], BF16, tag="dw")
                    nch = len(ecs)
                    nc.sync.dma_start(
                        dwti[:, :nch, :nn],
                        dwt_d[k, :nch, :, :nn].rearrange("c p f -> p c f"),
                    )

                    for s in range(STEPS):
                        xcur, xnxt = xb[s % 2], xb[(s + 1) % 2]
                        # --- transpose x^T -> X_row for gather stationary ---
                        xrow = xrowp.tile([128, 128], BF16, tag="xrow")
                        if col0 + 128 <= nnode:
                            nc.sync.dma_start(
                                xrow[:], xcur[:, col0 : col0 + 128], transpose=True
                            )
                        else:
                            stage = xrowp.tile([128, 128], BF16, tag="stage")
                            nc.vector.tensor_copy(stage[:, :nn], xcur[:, cols])
                            nc.sync.dma_start(xrow[:], stage[:], transpose=True)

                        # --- gather x[dst], x[src] (transposed edge layout) ---
                        pxd = pp350.tile([128, NE], F32, tag="p350")
                        pxs = pp350.tile([128, NE], F32, tag="p350")
                        nc.tensor.matmul(
                            pxd[:, :ne], xrow[:nn, :], sd[:nn, :ne],
                            start=True, stop=True,
                        )
                        nc.tensor.matmul(
                            pxs[:, :ne], xrow[:nn, :], sd[:nn, NE : NE + ne],
                            start=True, stop=True,
                        )
                        xdstT = eactp.tile([128, NE], BF16, tag="xdst")
                        xsrcT = eactp.tile([128, NE], BF16, tag="xsrc")
                        nc.scalar.copy(xdstT[:, :ne], pxd[:, :ne])
                        nc.vector.tensor_copy(xsrcT[:, :ne], pxs[:, :ne])

                        # --- W1: m1 = tanh(W1a.T xdst + W1b.T xsrc + b1) ---
                        m1 = eactp.tile([128, 2, NE], BF16, tag="m1")
                        for mc in range(2):
                            pm = pp350.tile([128, NE], F32, tag="p350")
                            nc.tensor.matmul(
                                pm[:, :ne],
                                w1_t[:, s, 0, mc * 128 : (mc + 1) * 128],
                                xdstT[:, :ne],
                                start=True, stop=False,
                            )
                            nc.tensor.matmul(
                                pm[:, :ne],
                                w1_t[:, s, 1, mc * 128 : (mc + 1) * 128],
                                xsrcT[:, :ne],
                                start=False, stop=True,
                            )
                            nc.scalar.activation(
                                m1[:, mc, :ne], pm[:, :ne], AF.Tanh,
                                bias=bcol(COL_B1(s, mc)),
                            )

                        # --- W2 ---
                        m2 = eactp.tile([128, 2, NE], BF16, tag="m2")
                        for mc in range(2):
                            pm = pp350.tile([128, NE], F32, tag="p350")
                            for kc in range(2):
                                nc.tensor.matmul(
                                    pm[:, :ne],
                                    w2_t[:, s, kc, mc * 128 : (mc + 1) * 128],
                                    m1[:, kc, :ne],
                                    start=(kc == 0), stop=(kc == 1),
                                )
                            nc.scalar.activation(
                                m2[:, mc, :ne], pm[:, :ne], AF.Tanh,
                                bias=bcol(COL_B2(s, mc)),
                            )

                        # --- W3 in row form (per edge chunk), no bias (folded) ---
                        pm3 = ppm3.tile([128, 384], F32, tag="pm3")
                        m3r = eactp.tile([128, 3, 128], BF16, tag="m3r")
                        for ci, (e0, el) in enumerate(ecs):
                            for kc in range(2):
                                nc.tensor.matmul(
                                    pm3[:el, ci * 128 : ci * 128 + 128],
                                    m2[:, kc, e0 : e0 + el],
                                    w3_t[:, s, kc, :],
                                    start=(kc == 0), stop=(kc == 1),
                                )
                            nc.vector.tensor_copy(
                                m3r[:el, ci, :], pm3[:el, ci * 128 : ci * 128 + 128]
                            )

                        # --- scatter: aggr^T = sum_e m3[e,:] * Dw[e,n] ---
                        pagg = ppagg.tile([128, NN], F32, tag="pagg")
                        for ci, (e0, el) in enumerate(ecs):
                            nc.tensor.matmul(
                                pagg[:, :nn],
                                m3r[:el, ci, :],
                                dwti[:el, ci, :nn],
                                start=(ci == 0), stop=(ci == len(ecs) - 1),
                            )
                        aggrT = gactp.tile([128, NN], BF16, tag="aggr")
                        nc.vector.tensor_copy(aggrT[:, :nn], pagg[:, :nn])

                        # --- GRU gate preactivations ---
                        pg = ppgru.tile([128, 504], F32, tag="pgru")
                        for g in range(2):  # r, z chunks: gi+gh accumulated
                            nc.tensor.matmul(
                                pg[:, 126 * g : 126 * g + nn],
                                wi_t[:, s, g * 128 : (g + 1) * 128],
                                aggrT[:, :nn],
                                start=True, stop=False,
                            )
                            nc.tensor.matmul(
                                pg[:, 126 * g : 126 * g + nn],
                                wh_t[:, s, g * 128 : (g + 1) * 128],
                                xcur[:, cols],
                                start=False, stop=True,
                            )
                        nc.tensor.matmul(
                            pg[:, 252 : 252 + nn],
                            wi_t[:, s, 256:384],
                            aggrT[:, :nn],
                            start=True, stop=True,
                        )
                        nc.tensor.matmul(
                            pg[:, 378 : 378 + nn],
                            wh_t[:, s, 256:384],
                            xcur[:, cols],
                            start=True, stop=True,
                        )
                        # th_r = tanh(0.5*(gi_r+gh_r+br)); r = 0.5*(1+th_r)
                        thr = gactp.tile([128, NN], BF16, tag="thr")
                        thz = gactp.tile([128, NN], BF16, tag="thz")
                        nc.scalar.activation(
                            thr[:, :nn], pg[:, :nn], AF.Tanh,
                            bias=bcol(COL_BRZ(s, 0)), scale=0.5,
                        )
                        nc.scalar.activation(
                            thz[:, :nn], pg[:, 126 : 126 + nn], AF.Tanh,
                            bias=bcol(COL_BRZ(s, 1)), scale=0.5,
                        )
                        # hn' = 0.5*(gh_n + bh_n); rhn = (1+th_r)*hn' = r*hn
                        hnp = gactp.tile([128, NN], BF16, tag="hnp")
                        nc.vector.tensor_scalar(
                            hnp[:, :nn], pg[:, 378 : 378 + nn],
                            bcol(COL_BHN(s)), 0.5, op0=ALU.add, op1=ALU.mult,
                        )
                        rhn = gactp.tile([128, NN], BF16, tag="rhn")
                        nc.vector.scalar_tensor_tensor(
                            rhn[:, :nn], thr[:, :nn], 1.0, hnp[:, :nn],
                            op0=ALU.add, op1=ALU.mult,
                        )
                        # n = tanh(gi_n + bi_n + rhn)
                        tn = gactp.tile([128, NN], BF16, tag="tn")
                        nc.vector.scalar_tensor_tensor(
                            tn[:, :nn], pg[:, 252 : 252 + nn],
                            bcol(COL_BIN(s)), rhn[:, :nn],
                            op0=ALU.add, op1=ALU.add,
                        )
                        ngate = gactp.tile([128, NN], BF16, tag="ng")
                        nc.scalar.activation(ngate[:, :nn], tn[:, :nn], AF.Tanh)
                        # x' = 0.5*((x+n) + th_z*(x-n))
                        a_ = gactp.tile([128, NN], BF16, tag="a")
                        d_ = gactp.tile([128, NN], BF16, tag="d")
                        nc.vector.tensor_add(a_[:, :nn], xcur[:, cols], ngate[:, :nn])
                        nc.vector.tensor_sub(d_[:, :nn], xcur[:, cols], ngate[:, :nn])
                        f_ = gactp.tile([128, NN], BF16, tag="f")
                        nc.vector.tensor_mul(f_[:, :nn], thz[:, :nn], d_[:, :nn])
                        g_ = gactp.tile([128, NN], BF16, tag="g")
                        nc.vector.tensor_add(g_[:, :nn], a_[:, :nn], f_[:, :nn])
                        nc.vector.tensor_scalar_mul(xnxt[:, cols], g_[:, :nn], 0.5)

            # ---------------- decoder ----------------
            xfin = xb[STEPS % 2]
            with (
                tc.tile_pool(name="decps", bufs=2, space=MemorySpace.PSUM) as decps,
                tc.tile_pool(name="d3ps", bufs=2, space=MemorySpace.PSUM) as d3psp,
                tc.tile_pool(name="dact", bufs=2) as dactp,
            ):
                for t0 in range(0, nnode, 512):
                    w = min(512, nnode - t0)
                    sl = slice(t0, t0 + w)
                    d1 = dactp.tile([128, 2, 512], BF16, tag="d1")
                    for mc in range(2):
                        ps = decps.tile([128, 512], F32, tag="dps")
                        nc.tensor.matmul(
                            ps[:, :w],
                            dw1_t[:, mc * 128 : (mc + 1) * 128],
                            xfin[:, sl],
                            start=True, stop=True,
                        )
                        nc.scalar.activation(
                            d1[:, mc, :w], ps[:, :w], AF.Tanh, bias=bcol(COL_DB1(mc))
                        )
                    d2 = dactp.tile([128, 2, 512], BF16, tag="d2")
                    for mc in range(2):
                        ps = decps.tile([128, 512], F32, tag="dps")
                        for kc in range(2):
                            nc.tensor.matmul(
                                ps[:, :w],
                                dw2_t[:, kc, mc * 128 : (mc + 1) * 128],
                                d1[:, kc, :w],
                                start=(kc == 0), stop=(kc == 1),
                            )
                        nc.scalar.activation(
                            d2[:, mc, :w], ps[:, :w], AF.Tanh, bias=bcol(COL_DB2(mc))
                        )
                    ps3 = d3psp.tile([1, 512], F32, tag="d3ps")
                    for kc in range(2):
                        nc.tensor.matmul(
                            ps3[:, :w], dw3_t[:, kc, :], d2[:, kc, :w],
                            start=(kc == 0), stop=(kc == 1),
                        )
                    nc.scalar.activation(
                        outsb[:, sl], ps3[:, :w], AF.Identity,
                        bias=bias_t[0:1, COL_DB3 : COL_DB3 + 1],
                    )
            nc.sync.dma_start(out_d[:], outsb[:])

    nc.compile()
    _NC_CACHE[key] = nc
    return nc


def preprocess(inputs, cfg):
    bg, nblk, tailg, totblk, nnode = (
        cfg["bg"], cfg["nblk"], cfg["tailg"], cfg["totblk"], cfg["nnode"],
    )
    b = bg * NCORES
    obs = np.asarray(inputs["obs"], np.float32)
    edges = np.asarray(inputs["edges"], np.int64)

    # one-hot incidence per graph
    src = edges[:, 0, :]
    dst = edges[:, 1, :]
    loops = np.broadcast_to(np.arange(N, dtype=np.int64), (b, N))
    src_all = np.concatenate([src, loops], 1)  # [b, 25]
    dst_all = np.concatenate([dst, loops], 1)
    nod = np.arange(N, dtype=np.int64)
    Sg = (src_all[:, None, :] == nod[None, :, None]).astype(np.float32)  # [b,9,25]
    Dg = (dst_all[:, None, :] == nod[None, :, None]).astype(np.float32)  # [b,9,25]
    deg = Dg.sum(2)  # [b, 9] >= 1
    Dw = Dg.transpose(0, 2, 1) / deg[:, None, :]  # [b, 25, 9]

    SDt = np.zeros((NCORES, totblk, NN, 2 * NE), NPBF16)
    DWf = np.zeros((NCORES, totblk, 384, NN), np.float32)
    Sg_ = Sg.reshape(NCORES, bg, N, EPG)
    Dg_ = Dg.reshape(NCORES, bg, N, EPG)
    Dw_ = Dw.reshape(NCORES, bg, EPG, N)
    nmain = nblk * GPB
    Sm = Sg_[:, :nmain].reshape(NCORES, nblk, GPB, N, EPG)
    Dm = Dg_[:, :nmain].reshape(NCORES, nblk, GPB, N, EPG)
    Wm = Dw_[:, :nmain].reshape(NCORES, nblk, GPB, EPG, N)
    for i in range(GPB):
        r = slice(N * i, N * i + N)
        c = slice(EPG * i, EPG * i + EPG)
        SDt[:, :nblk, r, c] = Dm[:, :, i]  # dst-gather one-hot
        SDt[:, :nblk, r, NE + EPG * i : NE + EPG * i + EPG] = Sm[:, :, i]
        DWf[:, :nblk, c, r] = Wm[:, :, i]
    for i in range(tailg):
        g = nmain + i
        r = slice(N * i, N * i + N)
        c = slice(EPG * i, EPG * i + EPG)
        SDt[:, nblk, r, c] = Dg_[:, g]
        SDt[:, nblk, r, NE + EPG * i : NE + EPG * i + EPG] = Sg_[:, g]
        DWf[:, nblk, c, r] = Dw_[:, g]
    DWt = DWf.reshape(NCORES, totblk, 3, 128, NN).astype(NPBF16)

    obsT = (
        obs.reshape(b, N, F_IN)
        .reshape(NCORES, nnode, F_IN)
        .transpose(0, 2, 1)
        .astype(NPBF16)
    )  # [8, 15, nnode]

    f32 = lambda x: np.asarray(x, np.float32)
    bf = lambda x: np.ascontiguousarray(f32(x)).astype(NPBF16)

    biases = np.zeros((128, NBIAS), np.float32)
    biases[:, COL_ENC] = f32(inputs["enc_b"])
    gru_bi = f32(inputs["gru_bi"])
    gru_bh = f32(inputs["gru_bh"])
    msg_b3 = f32(inputs["msg_b3"])
    gru_Wi = f32(inputs["gru_Wi"])
    for s in range(STEPS):
        b1 = f32(inputs["msg_b1"][s])
        b2 = f32(inputs["msg_b2"][s])
        for c in range(2):
            biases[:, COL_B1(s, c)] = b1[128 * c : 128 * (c + 1)]
            biases[:, COL_B2(s, c)] = b2[128 * c : 128 * (c + 1)]
        bip = gru_bi[s] + msg_b3[s] @ gru_Wi[s]  # fold msg_b3 into GRU input bias
        for g in range(2):
            biases[:, COL_BRZ(s, g)] = 0.5 * (
                bip[128 * g : 128 * (g + 1)] + gru_bh[s][128 * g : 128 * (g + 1)]
            )
        biases[:, COL_BHN(s)] = gru_bh[s][256:384]
        biases[:, COL_BIN(s)] = bip[256:384]
    db1 = f32(inputs["dec_b1"])
    db2 = f32(inputs["dec_b2"])
    for c in range(2):
        biases[:, COL_DB1(c)] = db1[128 * c : 128 * (c + 1)]
        biases[:, COL_DB2(c)] = db2[128 * c : 128 * (c + 1)]
    biases[0, COL_DB3] = float(f32(inputs["dec_b3"])[0])

    shared = dict(
        encw=bf(inputs["enc_W"]),
        w1=bf(inputs["msg_W1"]),
        w2=bf(inputs["msg_W2"]),
        w3=bf(inputs["msg_W3"]),
        wi=bf(inputs["gru_Wi"]),
        wh=bf(inputs["gru_Wh"]),
        dw1=bf(inputs["dec_W1"]),
        dw2=bf(inputs["dec_W2"]),
        dw3=bf(inputs["dec_W3"]),
        biases=biases,
    )
    in_maps = []
    for c in range(NCORES):
        m = dict(shared)
        m["obsT"] = np.ascontiguousarray(obsT[c])
        m["sdt"] = np.ascontiguousarray(SDt[c])
        m["dwt"] = np.ascontiguousarray(DWt[c])
        in_maps.append(m)
    return in_maps


LAST_EXEC_NS = None
TRACE = False


def _run(inputs, cfg):
    global LAST_EXEC_NS
    nc = build_nc(cfg)
    in_maps = preprocess(inputs, cfg)
    res = run_bass_kernel_spmd(
        nc, in_maps, core_ids=list(range(NCORES)), trace=TRACE
    )
    LAST_EXEC_NS = res.exec_time_ns
    bg = cfg["bg"]
    outs = [np.asarray(res.results[c]["out"], np.float32).reshape(bg, N) for c in range(NCORES)]
    full = np.concatenate(outs, 0)  # [B, 9]
    return np.ascontiguousarray(full[:, :8])


def kernel(**inputs) -> np.ndarray:
    return _run(inputs, CFG_FULL)
